# revision 4
# baseline (speedup 1.0000x reference)
"""Trainium2 Bass kernel for nn_CMAModel (memory-augmented causal attention).

v2: chunk-pipelined schedule tuned for the PE HAM clock gate.

Sharding: 8 cores = 2 batches x 4 head-groups; per core 4 heads as two
pairs (mq=0: heads 0,1 on partitions 0-63/64-127; mq=1: heads 2,3),
out_proj row-parallel, per-batch partials summed on host.

All matmul operands bf16 (FWL weight loads, half DMA). Schedule:
  phase A: proj chunk 0, memory k/V, gate logits+sigmoid for all chunks
  per chunk j: attention slots for both pairs, with proj of chunk j+1 and
    out-proj of chunk j-1 interleaved as PE fillers so the Tensor engine
    never idles (keeps the HAM clock gate at 8/8 = 2.4 GHz); one scalar
    Exp per slot covers both heads' score tiles (two PSUM banks, one
    instruction); combine evacuates accumulators to SBUF promptly to
    release PSUM, then normalizes via DMA partition-broadcast of 1/Z and
    g/Z; depthwise conv (residual folded into tap 3) on vector engine.
"""
import contextlib
import ctypes
import os
import sys
import types

import numpy as np

# ---------------------------------------------------------------- constants
B, T, C = 2, 2048, 1024
H, HD = 16, 64
M = 256
G = 4                 # head-groups (cores per batch)
HPG = H // G          # 4 heads per core
CPG = HPG * HD        # 256 channels per core
S = T + 2 * M         # 2560 kv rows
SM = 2 * M            # 512 memory rows
NKT = C // 128        # 8 contraction tiles
TC = 512              # T chunk size
NTC = T // TC         # 4
KCONV = 4
SCALE = 1.0 / float(np.sqrt(HD))

# broadcast path for 1/Z rows: "sbuf" = SBUF->SBUF DMA partition_broadcast,
# "dram" = bounce through DRAM (baseline-proven)
_BCAST = os.environ.get("BASS_BCAST", "dram")
_FILLERS_PER_SLOT = int(os.environ.get("BASS_FPS", "1"))

_BUILT = None


# ------------------------------------------------------- axon NTFF hook shim
def _install_ntff_hook():
    if "antenv.axon_hooks" in sys.modules:
        return
    so_path = "/opt/axon/libaxon_pjrt.so"
    hook = None
    if os.path.exists(so_path):
        try:
            lib = ctypes.CDLL(so_path)
            if hasattr(lib, "axon_start_nrt_profile"):
                lib.axon_start_nrt_profile.argtypes = [
                    ctypes.POINTER(ctypes.c_int64),
                    ctypes.c_size_t,
                ]
                lib.axon_start_nrt_profile.restype = ctypes.c_int64
                lib.axon_stop_nrt_profile.argtypes = [ctypes.c_char_p]
                lib.axon_stop_nrt_profile.restype = ctypes.c_int64

                @contextlib.contextmanager
                def _hook(output_dir, device_ids):
                    import jax

                    jax.devices()
                    if device_ids:
                        ids = (ctypes.c_int64 * len(device_ids))(*device_ids)
                        rc = lib.axon_start_nrt_profile(ids, len(device_ids))
                    else:
                        rc = lib.axon_start_nrt_profile(None, 0)
                    if rc != 0:
                        raise RuntimeError(f"axon_start_nrt_profile rc={rc}")
                    try:
                        yield
                    finally:
                        n = lib.axon_stop_nrt_profile(str(output_dir).encode())
                        if n < 0:
                            raise RuntimeError(f"axon_stop_nrt_profile rc={n}")

                hook = _hook
        except OSError:
            pass
    mod = types.ModuleType("antenv.axon_hooks")
    mod.get_axon_ntff_profile_hook = lambda: hook
    mod.set_axon_ntff_profile_hook = lambda h: None
    sys.modules["antenv.axon_hooks"] = mod


# ------------------------------------------------------------- device build
def _build_program():
    import concourse.tile as tile
    from concourse import bacc, mybir
    from concourse.masks import make_upper_triangular

    f32 = mybir.dt.float32
    bf16 = mybir.dt.bfloat16
    Exp = mybir.ActivationFunctionType.Exp
    mult = mybir.AluOpType.mult
    add = mybir.AluOpType.add

    nc = bacc.Bacc("TRN2", target_bir_lowering=False, debug=False, num_devices=8)

    xT = nc.dram_tensor("xT", [C, T], bf16, kind="ExternalInput").ap()
    memT = nc.dram_tensor("memT", [C, SM], bf16, kind="ExternalInput").ap()
    WqT = nc.dram_tensor("WqT", [C, CPG], bf16, kind="ExternalInput").ap()
    WkT = nc.dram_tensor("WkT", [C, CPG], bf16, kind="ExternalInput").ap()
    WvTa = nc.dram_tensor("WvTa", [C, 65 * HPG], bf16, kind="ExternalInput").ap()
    WgT = nc.dram_tensor("WgT", [C, HPG], bf16, kind="ExternalInput").ap()
    gbn = nc.dram_tensor("gbn", [HPG, 1], f32, kind="ExternalInput").ap()
    WoT = nc.dram_tensor("WoT", [CPG, C], bf16, kind="ExternalInput").ap()
    cw = nc.dram_tensor("cw", [CPG, KCONV], f32, kind="ExternalInput").ap()
    cb = nc.dram_tensor("cb", [CPG, 1], f32, kind="ExternalInput").ap()
    out = nc.dram_tensor("out", [T, C], bf16, kind="ExternalOutput").ap()
    outr = out.rearrange("t (b n) -> t b n", b=2)

    with tile.TileContext(nc) as tc:
        with contextlib.ExitStack() as ctx:
            const = ctx.enter_context(tc.tile_pool(name="const", bufs=1))
            sb = ctx.enter_context(tc.tile_pool(name="sb", bufs=1))
            work = ctx.enter_context(tc.tile_pool(name="work", bufs=1))
            psum = ctx.enter_context(
                tc.tile_pool(name="psum", bufs=1, space="PSUM")
            )
            drs = ctx.enter_context(
                tc.tile_pool(name="drs", bufs=4, space="DRAM")
            )

            # ---- weights + inputs (DMA order = first-use order)
            wq_s = const.tile([128, NKT, CPG], bf16)
            wk_s = const.tile([128, NKT, CPG], bf16)
            WqTr = WqT.rearrange("(a p) n -> p a n", p=128)
            WkTr = WkT.rearrange("(a p) n -> p a n", p=128)
            for m in range(2):
                cs = slice(m * 128, (m + 1) * 128)
                nc.sync.dma_start(out=wq_s[:, :, cs], in_=WqTr[:, :, cs])
                nc.sync.dma_start(out=wk_s[:, :, cs], in_=WkTr[:, :, cs])
            xs = sb.tile([128, NKT, T], bf16)
            xTr = xT.rearrange("(a p) t -> p a t", p=128)
            nc.sync.dma_start(out=xs[:, 0:2, 0:TC], in_=xTr[:, 0:2, 0:TC])
            nc.sync.dma_start(out=xs[:, 2:NKT, 0:TC], in_=xTr[:, 2:NKT, 0:TC])
            wva_s = const.tile([128, NKT, 65 * HPG], bf16)
            nc.sync.dma_start(out=wva_s, in_=WvTa.rearrange("(a p) n -> p a n", p=128))
            mems = sb.tile([128, NKT, SM], bf16)
            nc.sync.dma_start(out=mems, in_=memT.rearrange("(a p) t -> p a t", p=128))
            wg_s = const.tile([128, NKT, HPG], bf16)
            nc.sync.dma_start(out=wg_s, in_=WgT.rearrange("(a p) n -> p a n", p=128))
            for c in range(1, NTC):
                cs = slice(c * TC, (c + 1) * TC)
                nc.sync.dma_start(out=xs[:, :, cs], in_=xTr[:, :, cs])
            wo_s = const.tile([128, 2, C], bf16)
            nc.sync.dma_start(out=wo_s, in_=WoT.rearrange("(a p) n -> p a n", p=128))
            cw_s = const.tile([128, 2, KCONV], f32)
            nc.sync.dma_start(out=cw_s, in_=cw.rearrange("(a p) n -> p a n", p=128))
            cb_s = const.tile([128, 2, 1], f32)
            nc.sync.dma_start(out=cb_s, in_=cb.rearrange("(a p) n -> p a n", p=128))
            gbn_s = const.tile([HPG, 1], f32)
            nc.sync.dma_start(out=gbn_s, in_=gbn)

            # causal mask for diagonal blocks, duplicated for both heads
            tri = const.tile([128, 128], f32)
            make_upper_triangular(nc, tri, val=1.0, diag=True)
            tri2 = const.tile([128, 2, 128], f32)
            nc.vector.tensor_copy(tri2[:, 0, :], tri)
            nc.vector.tensor_copy(tri2[:, 1, :], tri)

            qT_s = sb.tile([128, 2, T], bf16)
            kT_s = sb.tile([128, 2, S], bf16)
            V_s = sb.tile([128, S // 128, 65 * HPG], bf16)
            # e = exp(-(logit+bias)) per head; sigmoid is folded into the
            # combine's reciprocal: g/Z = 1/(Z + Z*e)
            gE = sb.tile([HPG, T], f32)
            # e rows reshaped to 128 partitions: [128, head-in-pair, pair,
            # NTC*4] so combine ops run full-width
            ew = sb.tile([128, 2, 2, NTC * 4], f32)
            Y_s = sb.tile([128, 2, T], bf16)
            R_s = sb.tile([128, 2, T], bf16)

            # ---------------- building blocks -------------------------
            def proj_qk(ws, dst, mq, tglob, src, sloc, n):
                """dst[:, mq, tglob:tglob+n] = ws[:,:,mq-half].T @ src cols."""
                ps = psum.tile([128, 2, TC], f32, tag="ps", bufs=2, name="psq")
                for k in range(NKT):
                    nc.tensor.matmul(
                        ps[:, 0, :n],
                        ws[:, k, mq * 128:(mq + 1) * 128],
                        src[:, k, sloc:sloc + n],
                        start=(k == 0),
                        stop=(k == NKT - 1),
                    )
                nc.vector.tensor_copy(dst[:, mq, tglob:tglob + n], ps[:, 0, :n])

            def proj_v(st, src, sloc):
                """V_s[:, st, :] = src[:, :, sloc:+128].T @ WvTa (+ ones cols)."""
                ps = psum.tile([128, 2, TC], f32, tag="ps", bufs=2, name="psv")
                pv = ps[:, 0, : 65 * HPG]
                for k in range(NKT):
                    nc.tensor.matmul(
                        pv,
                        src[:, k, sloc:sloc + 128],
                        wva_s[:, k, :],
                        start=(k == 0),
                        stop=(k == NKT - 1),
                    )
                nc.vector.tensor_copy(V_s[:, st, :], pv)
                oc = V_s[:, st, 64:65 * HPG:65]
                nc.vector.tensor_scalar(oc, oc, 0.0, 1.0, mult, add)

            def proj_gate(cn):
                """gsig[:, chunk cn] = sigmoid(gate logits) via exp + 1/(1+e)."""
                tglob = cn * TC
                ps = psum.tile([128, 2, TC], f32, tag="ps", bufs=2, name="psg")
                pg = ps[0:HPG, 0, :]
                for k in range(NKT):
                    nc.tensor.matmul(
                        pg,
                        wg_s[:, k, :],
                        xs[:, k, tglob:tglob + TC],
                        start=(k == 0),
                        stop=(k == NKT - 1),
                    )
                gsl = gE[:, tglob:tglob + TC]
                nc.scalar.activation(gsl, pg, Exp, bias=gbn_s, scale=-1.0)
                # reshape e rows to 128-partition layout via DRAM bounce
                gdr = drs.tile([HPG, TC], f32, tag="gdr", bufs=2, name="gdr")
                nc.sync.dma_start(out=gdr, in_=gsl)
                for mq in range(2):
                    nc.sync.dma_start(
                        out=ew[:, :, mq, cn * 4:cn * 4 + 4],
                        in_=gdr[2 * mq:2 * mq + 2, :].rearrange(
                            "a (p c) -> p a c", p=128
                        ),
                    )

            def outproj_mt(mt, tail=False):
                """out rows [mt*128, +128) = R.T @ WoT, both 512-col halves."""
                po = psum.tile([128, 2, TC], f32, tag="ps", bufs=2, name="po")
                for nb in range(2):
                    for p in range(2):
                        nc.tensor.matmul(
                            po[:, nb, :],
                            R_s[:, p, mt * 128:(mt + 1) * 128],
                            wo_s[:, p, nb * TC:(nb + 1) * TC],
                            start=(p == 0),
                            stop=(p == 1),
                        )
                ot = work.tile([128, 2, TC], bf16, tag="ot", bufs=3, name="ot")
                nc.vector.tensor_copy(ot[:, 0, :], po[:, 0, :])
                nc.scalar.copy(ot[:, 1, :], po[:, 1, :])
                nc.sync.dma_start(
                    out=outr[mt * 128:(mt + 1) * 128, :, :], in_=ot
                )

            def conv_chunk(j, m, a=None, b=None):
                """causal depthwise conv K=4 (residual folded into tap 3)
                for one head-pair plane over columns [a, b)."""
                a = j * TC if a is None else a
                b = (j + 1) * TC if b is None else b
                # R = y*cw3' + cb  (cw3' = cw[3]+1 folds the residual)
                nc.vector.tensor_scalar(
                    R_s[:, m, a:b], Y_s[:, m, a:b],
                    cw_s[:, m, 3:4], cb_s[:, m, :], mult, add,
                )
                for k in range(KCONV - 1):
                    sh = KCONV - 1 - k
                    lo = max(a - sh, 0)
                    nc.vector.scalar_tensor_tensor(
                        R_s[:, m, lo + sh:b],
                        Y_s[:, m, lo:b - sh],
                        cw_s[:, m, k:k + 1],
                        R_s[:, m, lo + sh:b],
                        mult, add,
                    )

            # fillers: closures giving the scheduler dense PE work to slot
            # between attention matmuls
            filler_q = []

            def pop_fillers(nmax):
                for _ in range(nmax):
                    if not filler_q:
                        return
                    filler_q.pop(0)()

            def fill_q(cn):
                return [
                    (lambda mq=mq: proj_qk(wq_s, qT_s, mq, cn * TC, xs,
                                           cn * TC, TC))
                    for mq in range(2)
                ]

            def fill_k(cn):
                return [
                    (lambda mq=mq: proj_qk(wk_s, kT_s, mq, cn * TC, xs,
                                           cn * TC, TC))
                    for mq in range(2)
                ]

            def fill_v(cn):
                return [
                    (lambda st=cn * 4 + mt, sl=cn * TC + mt * 128:
                     proj_v(st, xs, sl))
                    for mt in range(TC // 128)
                ]

            def fill_o(cn):
                return [
                    (lambda mt=mt: outproj_mt(mt))
                    for mt in range(cn * 4, cn * 4 + 4)
                ]

            # ---------------- phase A ---------------------------------
            for mq in range(2):
                proj_qk(wq_s, qT_s, mq, 0, xs, 0, TC)
                proj_qk(wk_s, kT_s, mq, 0, xs, 0, TC)
            for mt in range(TC // 128):
                proj_v(mt, xs, mt * 128)
            for mq in range(2):          # memory keys -> kT_s[:, mq, T:]
                ps = psum.tile([128, 2, TC], f32, tag="ps", bufs=2, name="psm")
                for k in range(NKT):
                    nc.tensor.matmul(
                        ps[:, 0, :],
                        wk_s[:, k, mq * 128:(mq + 1) * 128],
                        mems[:, k, :],
                        start=(k == 0),
                        stop=(k == NKT - 1),
                    )
                nc.vector.tensor_copy(kT_s[:, mq, T:], ps[:, 0, :])
            for mt in range(SM // 128):  # memory values
                proj_v(T // 128 + mt, mems, mt * 128)
            proj_gate(0)

            # per-block filler lists: (early, paced). Early fillers pop one
            # per slot from slot 0 (chunk 3's k/V are read mid-block-3, so
            # they must land within the first ~10 slots). Paced fillers are
            # spread evenly across the block — front-loading makes late
            # slots PE-starved and pops conv-dependent out-proj work before
            # the vector queue has drained.
            block_fillers = [
                ([], fill_q(1) + fill_k(1) + fill_v(1)
                 + [lambda: proj_gate(1)]),
                ([], fill_q(2) + fill_k(2) + fill_v(2)
                 + [lambda: proj_gate(2)] + fill_o(0)),
                ([], fill_q(3) + [lambda: proj_gate(3)] + fill_o(1)),
                (fill_k(3) + fill_v(3), fill_o(2)[:2]),
            ]
            tail_fillers = fill_o(2)[2:]

            # ---------------- attention chunks ------------------------
            for j in range(NTC):
                early_q, paced = block_fillers[j]
                filler_q.extend(paced)
                nct = 4 * (j + 1)
                slots_total = 2 * (nct + 4)
                nfill = len(filler_q)
                slot_ctr = popped = 0
                for mq in range(2):
                    acc = [
                        psum.tile([128, TC], f32, tag="pa", bufs=4,
                                  name=f"acc{mq}{j}{x}")
                        for x in range(4)           # AcA, AmA, AcB, AmB
                    ]
                    for i in range(nct + 4):
                        is_mem = i >= nct
                        si = (T // 128 + i - nct) if is_mem else i
                        off = 0
                        if not is_mem and si >= 4 * j:
                            off = 128 * si - TC * j
                        ps = psum.tile([128, 2, TC], f32, tag="ps", bufs=2,
                                       name="pss")
                        for a in range(2):
                            nc.tensor.matmul(
                                ps[:, a, off:],
                                kT_s[64 * a:64 * a + 64, mq,
                                     si * 128:(si + 1) * 128],
                                qT_s[64 * a:64 * a + 64, mq,
                                     TC * j + off:TC * (j + 1)],
                                start=True,
                                stop=True,
                            )
                        Pt = work.tile([128, 2, TC], bf16, tag="P", bufs=3,
                                       name="Pt")
                        nc.scalar.activation(
                            Pt[:, :, off:], ps[:, :, off:], Exp, scale=SCALE
                        )
                        if not is_mem and si >= 4 * j:
                            doff = 128 * si - TC * j
                            nc.vector.tensor_mul(
                                Pt[:, :, doff:doff + 128],
                                Pt[:, :, doff:doff + 128],
                                tri2,
                            )
                        # PE filler between QK and PV: the PV waits on the
                        # scalar exp, so give the PE independent work here
                        slot_ctr += 1
                        if early_q:
                            early_q.pop(0)()
                        while popped < (slot_ctr * nfill) // slots_total:
                            pop_fillers(1)
                            popped += 1
                        for a in range(2):
                            dst = acc[2 * a + (1 if is_mem else 0)]
                            first = (i == 0) or (is_mem and i == nct)
                            last = (i == nct - 1) or (i == nct + 3)
                            nc.tensor.matmul(
                                dst[0:65, off:],
                                V_s[:, si, 65 * (2 * mq + a):
                                    65 * (2 * mq + a) + 65],
                                Pt[:, a, off:],
                                start=first,
                                stop=last,
                            )

                    # ---- combine: evacuate PSUM, normalize, gate ----
                    # (DVE ops need all operands on the same partitions, so
                    # Z math stays on partition 64, DMA aligns rows, and
                    # head B's result is DMA-moved into partitions 64-127)
                    acs = work.tile([128, 4, TC], f32, tag="acs", bufs=2,
                                    name="acs")
                    for x in range(4):
                        nc.vector.tensor_copy(acs[0:65, x, :], acc[x][0:65, :])
                    # Z rows -> DRAM -> [128, a, {Zc,Zm}, 4] full-width layout
                    zdraw = drs.tile([4, TC], f32, tag="zdraw", bufs=4,
                                     name="zdraw")
                    nc.sync.dma_start(out=zdraw, in_=acs[64:65, :, :])
                    zw = work.tile([128, 2, 2, 4], f32, tag="zw", bufs=2,
                                   name="zw")
                    for a in range(2):
                        nc.sync.dma_start(
                            out=zw[:, a, :, :],
                            in_=zdraw[2 * a:2 * a + 2, :].rearrange(
                                "q (p c) -> p q c", p=128
                            ),
                        )
                    # zf plane 0 = 1/Z, plane 1 = g/Z = 1/(Z + Z*e)
                    zf = work.tile([128, 2, 2, 4], f32, tag="zf", bufs=2,
                                   name="zf")
                    nc.vector.tensor_add(
                        zf[:, :, 0, :], zw[:, :, 0, :], zw[:, :, 1, :]
                    )
                    nc.vector.tensor_mul(
                        zf[:, :, 1, :], zf[:, :, 0, :],
                        ew[:, :, mq, 4 * j:4 * (j + 1)],
                    )
                    nc.vector.tensor_add(
                        zf[:, :, 1, :], zf[:, :, 1, :], zf[:, :, 0, :]
                    )
                    nc.vector.reciprocal(zf, zf)
                    zdr = drs.tile([1, 2, 2, TC], f32, tag="zdr", bufs=4,
                                   name="zdr")
                    for a in range(2):
                        nc.sync.dma_start(
                            out=zdr[0:1, a, :, :].rearrange(
                                "o q (p c) -> (o p) q c", p=128
                            ),
                            in_=zf[:, a, :, :],
                        )
                    # bcast planes: [64, head, rz/gz, TC], all at parts 0-63
                    bcast = work.tile([64, 2, 2, TC], f32, tag="bc", bufs=2,
                                      name="bcast")
                    nc.sync.dma_start(
                        out=bcast, in_=zdr.partition_broadcast(64)
                    )
                    ytmp = work.tile([64, 2, TC], f32, tag="yt", bufs=2,
                                     name="ytmp")
                    for a in range(2):
                        nc.vector.tensor_mul(
                            ytmp[:, 0, :], acs[0:64, 2 * a, :],
                            bcast[:, a, 0, :],
                        )
                        nc.vector.tensor_mul(
                            ytmp[:, 1, :], acs[0:64, 2 * a + 1, :],
                            bcast[:, a, 1, :],
                        )
                        if a == 0:
                            nc.vector.tensor_add(
                                Y_s[0:64, mq, TC * j:TC * (j + 1)],
                                ytmp[:, 0, :], ytmp[:, 1, :],
                            )
                        else:
                            yb = work.tile([64, TC], bf16, tag="yb", bufs=2,
                                           name="yb")
                            nc.vector.tensor_add(yb, ytmp[:, 0, :],
                                                 ytmp[:, 1, :])
                            nc.sync.dma_start(
                                out=Y_s[64:128, mq, TC * j:TC * (j + 1)],
                                in_=yb,
                            )
                if j < NTC - 1:
                    conv_chunk(j, 0)
                    conv_chunk(j, 1)
                else:
                    # tail: keep the PE busy with chunk-2 out-proj while the
                    # last combine's DMA chain drains, then conv the final
                    # chunk in 256-col pieces, starting each out-proj tile
                    # as soon as its columns are ready
                    pop_fillers(len(filler_q))
                    for f in tail_fillers:
                        f()
                    base = j * TC
                    for half in range(2):
                        lo, hi = base + 256 * half, base + 256 * (half + 1)
                        conv_chunk(j, 0, lo, hi)
                        conv_chunk(j, 1, lo, hi)
                        for mt in range(j * 4 + 2 * half,
                                        j * 4 + 2 * half + 2):
                            outproj_mt(mt)

    nc.compile()
    return nc


def _get_program():
    global _BUILT
    if _BUILT is None:
        _install_ntff_hook()
        _BUILT = _build_program()
    return _BUILT


# --------------------------------------------------------------- host side
def _bf16(a):
    import ml_dtypes

    return np.ascontiguousarray(np.asarray(a, np.float32)).astype(
        ml_dtypes.bfloat16
    )


def host_prep(inputs):
    x = np.asarray(inputs["x"], np.float32)
    fwd = np.asarray(inputs["fwd_mem"], np.float32)
    rev = np.asarray(inputs["rev_mem"], np.float32)
    Wq = np.asarray(inputs["Wq"], np.float32)
    Wk = np.asarray(inputs["Wk"], np.float32)
    Wv = np.asarray(inputs["Wv"], np.float32)
    Wo = np.asarray(inputs["Wo"], np.float32)
    gate_w = np.asarray(inputs["gate_w"], np.float32)
    gate_b = np.asarray(inputs["gate_b"], np.float32)
    canon_w = np.asarray(inputs["canon_w"], np.float32)
    canon_bias = np.asarray(inputs["canon_bias"], np.float32)

    Wg = (gate_w.astype(np.float64) @ Wq.astype(np.float64)).astype(np.float32)

    per_b, per_g = [], []
    for b in range(B):
        per_b.append({
            "xT": _bf16(x[b].T),
            "memT": _bf16(np.concatenate([fwd[b], rev[b]], axis=0).T),
        })
    for g in range(G):
        cs = slice(g * CPG, (g + 1) * CPG)
        WvTa = np.zeros((C, 65 * HPG), np.float32)
        for h in range(HPG):
            rows = Wv[g * CPG + h * HD: g * CPG + (h + 1) * HD]
            WvTa[:, 65 * h:65 * h + 64] = rows.T
        hs = slice(g * HPG, (g + 1) * HPG)
        cwg = np.array(canon_w[cs, 0, :], np.float32, copy=True)
        cwg[:, KCONV - 1] += 1.0      # fold residual into last conv tap
        per_g.append({
            "WqT": _bf16(Wq[cs].T),
            "WkT": _bf16(Wk[cs].T),
            "WvTa": _bf16(WvTa),
            "WgT": _bf16(Wg[hs].T),
            "gbn": np.ascontiguousarray(-gate_b[hs]).reshape(HPG, 1),
            "WoT": _bf16(Wo[:, cs].T),
            "cw": cwg,
            "cb": np.ascontiguousarray(canon_bias[cs]).reshape(CPG, 1),
        })
    return per_b, per_g


LAST_EXEC_NS = None
LAST_RESULTS = None


def kernel(**inputs):
    global LAST_EXEC_NS, LAST_RESULTS
    from concourse.bass_utils import run_bass_kernel_spmd

    nc = _get_program()
    per_b, per_g = host_prep(inputs)
    in_maps = []
    for core in range(8):
        b, g = divmod(core, G)
        m = {}
        m.update(per_b[b])
        m.update(per_g[g])
        in_maps.append(m)

    trace = bool(int(os.environ.get("KERNEL_TRACE", "0")))
    kw = {}
    if trace:
        tcores = os.environ.get("KERNEL_TRACE_CORES", "0")
        kw = dict(
            trace=True,
            trace_cores=[int(c) for c in tcores.split(",")],
            tmpdir=os.environ.get("KERNEL_TRACE_DIR", None),
        )
    res = run_bass_kernel_spmd(nc, in_maps, core_ids=list(range(8)), **kw)
    LAST_EXEC_NS = res.exec_time_ns
    LAST_RESULTS = res
    outp = np.zeros((B, T, C), np.float32)
    for core in range(8):
        b = core // G
        outp[b] += np.asarray(res.results[core]["out"], np.float32)
    return outp
